# revision 37
# baseline (speedup 1.0000x reference)
"""Based linear-attention (parallel form) on 8 TRN2 NeuronCores.

Sharding: core c handles batch b = c // 4 and head-group g = c % 4
(3 of 12 heads).  Wq/Wk/Wv column-split by head, Wo row-split; each
core emits a partial [L, D] output and the host sums the 4 partials per
batch as part of unsharding.

v1 restructure vs v0 baseline (224us):
  - host pre-swizzles every weight into its SBUF layout so each input is
    ONE dma_start (issue cost ~0.6us each on the sync queue; v0 spent
    86us issuing 140 descriptors)
  - fused per-strip pipeline: V-proj group j -> attention strip j ->
    normalize -> output projection + store, all interleaved so PE never
    waits on a phase boundary and ACT/DVE copies spread across the run
  - ones row appended to qT/kT (K=17) so the PE emits s+1 directly;
    squares then split ACT (Square) / DVE (tensor_mul) per head to
    balance the two elementwise engines
  - off-diagonal tiles drop the "+1" fixup pass entirely: attn=(s+1)^2
    and the missing +1 is folded in as (o += cumsative v sums) via the
    normalize STT's per-partition scalar, and z += 128*(#full tiles)
  - z normalizer still via ones-lhsT matmuls (col-tiled across heads),
    1/z broadcast via K=1 matmul, normalize+drain fused in one DVE STT
"""

import sys

sys.path.insert(0, "/opt/trn_rl_repo")

from contextlib import ExitStack

import ml_dtypes
import numpy as np

import concourse.bass as bass
import concourse.tile as tile
from concourse import bacc, mybir
from concourse.bass_utils import run_bass_kernel_spmd

B, L, D = 2, 2048, 1536
H, FDIM, HD = 12, 16, 128
NH = 3          # heads per core
GQK = 96        # padded q/k rows (3 heads x 32; rows 32h..32h+15 = head,
                # row 32h+16 = ones so the K=17 matmul emits s+1)
DV = NH * HD    # 384 v/o columns per core
SW = 512        # l-strip width
P = 128
NK = D // P     # 12 contraction tiles
NM = L // P     # 16 m/l tiles
NJ = L // SW    # 4 l strips
NDC = D // SW   # 3 output column strips

DT = mybir.dt.bfloat16
NPDT = ml_dtypes.bfloat16
F32 = mybir.dt.float32
FP16 = mybir.dt.float16

_ADD = mybir.AluOpType.add
_MULT = mybir.AluOpType.mult
_SQUARE = mybir.ActivationFunctionType.Square
_COPY = mybir.ActivationFunctionType.Copy


def _build():
    nc = bacc.Bacc("TRN2", target_bir_lowering=False, debug=False, num_devices=8)

    hsT = nc.dram_tensor("hsT", [D, L], DT, kind="ExternalInput").ap()
    wq = nc.dram_tensor("wq", [P, NK * GQK], DT, kind="ExternalInput").ap()
    wk = nc.dram_tensor("wk", [P, NK * GQK], DT, kind="ExternalInput").ap()
    wv = nc.dram_tensor("wv", [P, NK * DV], DT, kind="ExternalInput").ap()
    wo = nc.dram_tensor("wo", [P, NH * D], DT, kind="ExternalInput").ap()
    masks = nc.dram_tensor("masks", [P, NJ * SW], DT, kind="ExternalInput").ap()
    out = nc.dram_tensor("out", [L, D], DT, kind="ExternalOutput").ap()

    with tile.TileContext(nc, trace_sim=False) as tc, ExitStack() as ctx:
        cpool = ctx.enter_context(tc.tile_pool(name="consts", bufs=1))
        wq_sb = cpool.tile([P, NK * GQK], DT, tag="wq")
        wk_sb = cpool.tile([P, NK * GQK], DT, tag="wk")
        wv_sb = cpool.tile([P, NK * DV], DT, tag="wv")
        wo_sb = cpool.tile([P, NH * D], DT, tag="wo")
        masks_sb = cpool.tile([P, NJ * SW], DT, tag="masks")
        ones_col = cpool.tile([P, 1], DT, tag="ones_col")
        ones_bc = cpool.tile([GQK, P], DT, tag="ones_bc")
        ones_rhs = cpool.tile([P, SW], DT, tag="ones_rhs")
        esel = cpool.tile([P, GQK], DT, tag="esel")
        vcum_sb = cpool.tile([P, NH], F32, tag="vcum")
        qT_sb = cpool.tile([GQK, L], DT, tag="qT")
        kT_sb = cpool.tile([GQK, L], DT, tag="kT")
        v_sb = cpool.tile([P, NM * DV], DT, tag="v")
        oT_sb = [cpool.tile([P, L], DT, tag=f"oT{h}", name=f"oT{h}") for h in range(NH)]
        zr_sb = cpool.tile([GQK, L], DT, tag="zr")
        zr32 = cpool.tile([GQK, SW], F32, tag="zr32")
        zsum_sb = cpool.tile([GQK, SW], F32, tag="zsum")
        bc_sb = [cpool.tile([P, SW], DT, tag=f"bc{h}", name=f"bc{h}") for h in range(NH)]
        hpool = ctx.enter_context(tc.tile_pool(name="hsT", bufs=NK))
        hs_t = [hpool.tile([P, L], DT, tag="hsT", name=f"hsT{k}") for k in range(NK)]

        # ---- input DMAs: one per tensor, hsT per k-strip for pipelining ----
        nc.sync.dma_start(wq_sb[:], wq[:])
        nc.sync.dma_start(hs_t[0][:], hsT[0:P, :])
        nc.sync.dma_start(wk_sb[:], wk[:])
        for k in range(1, 4):
            nc.sync.dma_start(hs_t[k][:], hsT[k * P : (k + 1) * P, :])
        nc.sync.dma_start(wv_sb[:], wv[:])
        for k in range(4, NK):
            nc.sync.dma_start(hs_t[k][:], hsT[k * P : (k + 1) * P, :])
        nc.sync.dma_start(masks_sb[:], masks[:])
        nc.sync.dma_start(wo_sb[:], wo[:])
        nc.gpsimd.memset(ones_col[:], 1.0)
        nc.gpsimd.memset(ones_bc[:], 1.0)
        nc.gpsimd.memset(vcum_sb[:], 0.0)
        nc.gpsimd.memset(ones_rhs[:], 1.0)
        # E-selector: E[0, 32h] = 1 -> an extra accumulating matmul
        # E.T @ ones sets qT/kT row 32h to 1.0 (the K=17 "s+1" trick)
        # without a slow single-partition memset after the drains
        nc.gpsimd.memset(esel[:], 0.0)
        for h in range(NH):
            nc.gpsimd.memset(esel[0:1, 32 * h : 32 * h + 1], 1.0)

        # ---- fused pipeline; QK proj shares the strip pools' psum so the
        #      attention matmuls aren't gated on a pool-boundary barrier ----
        with tc.tile_pool(name="ps_sq", bufs=NH, space="PSUM") as ps_sq, \
             tc.tile_pool(name="ps_acc", bufs=NH, space="PSUM") as ps_acc, \
             tc.tile_pool(name="ps_z", bufs=1, space="PSUM") as ps_z, \
             tc.tile_pool(name="ps_x", bufs=1, space="PSUM") as ps_x, \
             tc.tile_pool(name="attnT", bufs=15) as apool, \
             tc.tile_pool(name="obuf", bufs=3) as obuf:
            # -- Q/K projections, k-outer so matmuls chase the hsT DMAs --
            qps = [ps_sq.tile([GQK, SW], F32, tag="s", name=f"qp{j}") for j in range(NH)]
            qps.append(ps_z.tile([GQK, SW], F32, tag="z", name="qp3"))
            kps = [ps_acc.tile([GQK, SW], F32, tag="o", name=f"kp{j}") for j in range(NH)]
            kps.append(ps_x.tile([GQK, SW], F32, tag="x", name="kp3"))
            for k in range(NK):
                for j in range(NJ):
                    nc.tensor.matmul(
                        qps[j][:], wq_sb[:, k * GQK : (k + 1) * GQK],
                        hs_t[k][:, j * SW : (j + 1) * SW],
                        start=(k == 0), stop=False)
                for j in range(NJ):
                    nc.tensor.matmul(
                        kps[j][:], wk_sb[:, k * GQK : (k + 1) * GQK],
                        hs_t[k][:, j * SW : (j + 1) * SW],
                        start=(k == 0), stop=False)
            for j in range(NJ):
                nc.tensor.matmul(qps[j][:], esel[:], ones_rhs[:], start=False, stop=True)
                nc.tensor.matmul(kps[j][:], esel[:], ones_rhs[:], start=False, stop=True)
            # drain order frees the x- and z-banks first (they double-buffer
            # V-proj), then s (sT tiles); alternate ACT/DVE to halve latency
            drains = [(kps[3], kT_sb, 3), (qps[3], qT_sb, 3), (qps[0], qT_sb, 0),
                      (qps[1], qT_sb, 1), (qps[2], qT_sb, 2), (kps[0], kT_sb, 0),
                      (kps[1], kT_sb, 1), (kps[2], kT_sb, 2)]
            for idx, (src, dst, jj) in enumerate(drains):
                if idx % 2 == 0:
                    nc.scalar.copy(dst[:, jj * SW : (jj + 1) * SW], src[:])
                else:
                    nc.vector.tensor_copy(dst[:, jj * SW : (jj + 1) * SW], src[:])

            def emit_vproj(g):
                # double-buffer across the x- and z-rings (z is idle during
                # V-proj) so vp never waits on its own drain
                for n, mt in enumerate(range(4 * g, 4 * g + 4)):
                    pool, tg = (ps_x, "x") if n % 2 == 0 else (ps_z, "z")
                    vp = pool.tile([P, SW], F32, tag=tg, name=f"vp{mt}")
                    for k in range(NK):
                        nc.tensor.matmul(
                            vp[:, :DV], hs_t[k][:, mt * P : (mt + 1) * P],
                            wv_sb[:, k * DV : (k + 1) * DV],
                            start=(k == 0), stop=(k == NK - 1))
                    nc.vector.tensor_copy(v_sb[:, mt * DV : (mt + 1) * DV], vp[:, :DV])

            # oproj work from strip j is deferred and interleaved into strip
            # j+1's attention stream as PE ballast: it has no elementwise
            # dependencies, so it fills the PE's dependency micro-gaps and
            # keeps the HAM clock-gate at full rate (idle windows re-throttle
            # the PE to 1.2 GHz for ~7-14us at a time, measured)
            obtiles = {}

            def make_oproj_group(lt, dc, flip):
                def emit():
                    ops = ps_x.tile([P, SW], F32, tag="x", name=f"op{lt}_{dc}")
                    for h in range(NH):
                        nc.tensor.matmul(
                            ops[:], oT_sb[h][:, lt * P : (lt + 1) * P],
                            wo_sb[:, h * D + dc * SW : h * D + (dc + 1) * SW],
                            start=(h == 0), stop=(h == NH - 1))
                    if dc == 0:
                        obtiles[lt] = obuf.tile([P, D], DT, tag="ob", name=f"ob{lt}")
                    ob = obtiles[lt]
                    if flip:
                        nc.scalar.copy(ob[:, dc * SW : (dc + 1) * SW], ops[:])
                    else:
                        nc.vector.tensor_copy(ob[:, dc * SW : (dc + 1) * SW], ops[:])
                    if dc == NDC - 1:
                        nc.sync.dma_start(out[lt * P : (lt + 1) * P, :], ob[:])
                return emit

            ballast = []

            for j in range(NJ):
                if j == 0:
                    emit_vproj(0)

                # -- (b) attention strip j --
                nim = 4 * (j + 1)
                otp = [ps_acc.tile([P, SW], F32, tag="o", name=f"otp{j}_{h}")
                       for h in range(NH)]
                zp = ps_z.tile([GQK, SW], F32, tag="z", name=f"zp{j}")

                def emit_av(im, atts, j=j, nim=nim, otp=otp, zp=zp):
                    c = im - 4 * j
                    f0 = max(c, 0) * P
                    for h in range(NH):
                        nc.tensor.matmul(
                            otp[h][:, f0:SW],
                            v_sb[:, im * DV + h * HD : im * DV + (h + 1) * HD],
                            atts[h][:, f0:SW], start=(im == 0), stop=(im == nim - 1))
                    for h in range(NH):
                        nc.tensor.matmul(
                            zp[32 * h : 32 * h + 1, f0:SW], ones_col[:], atts[h][:, f0:SW],
                            start=(im == 0), stop=(im == nim - 1))

                pending = []   # (im, atts) awaiting AV/z; depth-2 so the PE
                               # never dispatch-stalls on a square in flight
                nsq = 0
                for im in range(nim):
                    c = im - 4 * j
                    f0 = max(c, 0) * P
                    cur = []
                    for h in range(NH):
                        r0 = 32 * h
                        stp = ps_sq.tile([P, SW], F32, tag="s",
                                        name=f"stp{j}_{im}_{h}")
                        nsq += 1
                        nc.tensor.matmul(
                            stp[:, f0:SW], kT_sb[r0 : r0 + FDIM + 1, im * P : (im + 1) * P],
                            qT_sb[r0 : r0 + FDIM + 1, j * SW + f0 : (j + 1) * SW],
                            start=True, stop=True)
                        att = apool.tile([P, SW], DT, tag="a")
                        if c >= 0:
                            # diag: ((s+1)^2 + 1) * mask   (ACT square, DVE fixup)
                            nc.scalar.activation(att[:, f0:SW], stp[:, f0:SW], _SQUARE)
                            nc.vector.scalar_tensor_tensor(
                                att[:, f0:SW], att[:, f0:SW], 1.0,
                                masks_sb[:, c * SW + f0 : (c + 1) * SW],
                                op0=_ADD, op1=_MULT)
                        elif h == 1:
                            # off-diag on DVE (drain cast + in-place square)
                            # to relieve ACT, the attention-loop bottleneck
                            nc.vector.tensor_copy(att[:], stp[:])
                            nc.vector.tensor_mul(att[:], att[:], att[:])
                        else:
                            # off-diag: (s+1)^2, +1 folded via vcum/z-const
                            nc.scalar.activation(att[:], stp[:], _SQUARE)
                        cur.append(att)
                    pending.append((im, cur))
                    if len(pending) > 2:
                        pim, patts = pending.pop(0)
                        emit_av(pim, patts)
                    # PE ballast: prior strip's oproj groups (start at slot 2
                    # so the normalize STTs they depend on have completed)
                    if im >= 2 and ballast:
                        ballast.pop(0)()
                        if len(ballast) > 2 * (nim - im - 1) and ballast:
                            ballast.pop(0)()
                for pim, patts in pending:
                    emit_av(pim, patts)
                while ballast:
                    ballast.pop(0)()

                # -- (c) normalizer + per-strip normalize --
                if j > 0:
                    # off-diag tiles' missing "+1": z gains 128 per full tile
                    nc.vector.tensor_scalar_add(zsum_sb[:], zp[:], float(P * 4 * j))
                    nc.vector.reciprocal_approx_fast(zr32[:], zsum_sb[:])
                else:
                    nc.vector.reciprocal_approx_fast(zr32[:], zp[:])
                with nc.allow_low_precision(reason="1/z in fp16 is plenty for 2e-2 gate"):
                    nc.vector.tensor_copy(zr_sb[:, j * SW : (j + 1) * SW], zr32[:])
                # next strip's V-proj keeps PE busy under the recip chain
                if j < NJ - 1:
                    emit_vproj(j + 1)
                for h in range(NH):
                    r0 = 32 * h
                    bc = ps_x.tile([P, SW], F32, tag="x", name=f"bc{j}_{h}")
                    nc.tensor.matmul(
                        bc[:], ones_bc[r0 : r0 + 1, :],
                        zr_sb[r0 : r0 + 1, j * SW : (j + 1) * SW],
                        start=True, stop=True)
                    bcs = bc_sb[h]
                    nc.vector.tensor_copy(bcs[:], bc[:])
                    # oT = (otp + vcum) * bc  : drain+(+1-fold)+normalize in one op
                    nc.vector.scalar_tensor_tensor(
                        oT_sb[h][:, j * SW : (j + 1) * SW], otp[h][:],
                        vcum_sb[:, h : h + 1], bcs[:], op0=_ADD, op1=_MULT)
                # vcum += column sums of this strip's v tiles (for NEXT strips)
                if j < NJ - 1:
                    vps = ps_x.tile([P, SW], F32, tag="x", name=f"vps{j}")
                    # accumulation groups must be sequential per PSUM bank:
                    # interleaving start/stop groups at different free offsets
                    # corrupts the accumulation (HW-verified)
                    for h in range(NH):
                        for mt in range(4 * j, 4 * j + 4):
                            nc.tensor.matmul(
                                vps[:, h : h + 1],
                                v_sb[:, mt * DV + h * HD : mt * DV + (h + 1) * HD],
                                ones_col[:], start=(mt == 4 * j), stop=(mt == 4 * j + 3))
                    nc.vector.tensor_add(vcum_sb[:], vcum_sb[:], vps[:, :NH])

                # -- (d) output projection: deferred into strip j+1's stream --
                flip = True
                for lt in range(4 * j, 4 * j + 4):
                    for dc in range(NDC):
                        ballast.append(make_oproj_group(lt, dc, flip))
                        flip = not flip
                if j == NJ - 1:
                    while ballast:
                        ballast.pop(0)()

    nc.compile()
    return nc


def _host_inputs(hidden_states, Wq, Wk, Wv, Wo):
    """Shard + lay out the full inputs into 8 per-core in_maps.

    All weights are pre-swizzled to their SBUF layouts so each is one DMA.
    """
    scale = FDIM ** -0.5
    mask = np.zeros((P, NJ * SW), dtype=np.float32)
    p = np.arange(P)[:, None]
    f = np.arange(SW)[None, :]
    for c in range(NJ):
        mask[:, c * SW : (c + 1) * SW] = (p + P * c <= f).astype(np.float32)

    in_maps = []
    for core in range(8):
        b, g = divmod(core, 4)
        heads = range(NH * g, NH * (g + 1))
        # [P, NK*GQK]: chunk k holds Wq[128k:128k+128, :] with head h at
        # columns 32h+1..32h+16 (rest zero; row 32h seeded to 1 on device
        # for the K=17 "s+1" trick — 1-partition ops need 32-aligned base)
        wq_pre = np.zeros((P, NK * GQK), dtype=np.float32)
        wk_pre = np.zeros((P, NK * GQK), dtype=np.float32)
        for k in range(NK):
            for i, h in enumerate(heads):
                wq_pre[:, k * GQK + 32 * i + 1 : k * GQK + 32 * i + 1 + FDIM] = \
                    Wq[k * P : (k + 1) * P, FDIM * h : FDIM * (h + 1)] * scale
                wk_pre[:, k * GQK + 32 * i + 1 : k * GQK + 32 * i + 1 + FDIM] = \
                    Wk[k * P : (k + 1) * P, FDIM * h : FDIM * (h + 1)]
        # [P, NK*DV]: chunk k = Wv[128k:128k+128, group cols]
        wv_pre = Wv[:, DV * g : DV * (g + 1)].reshape(NK, P, DV)
        wv_pre = wv_pre.transpose(1, 0, 2).reshape(P, NK * DV)
        # [P, NH*D]: chunk h = Wo[384g+128h : +128, :]
        wo_pre = Wo[DV * g : DV * (g + 1), :].reshape(NH, P, D)
        wo_pre = wo_pre.transpose(1, 0, 2).reshape(P, NH * D)
        in_maps.append({
            "hsT": np.ascontiguousarray(hidden_states[b].T).astype(NPDT),
            "wq": wq_pre.astype(NPDT),
            "wk": wk_pre.astype(NPDT),
            "wv": np.ascontiguousarray(wv_pre).astype(NPDT),
            "wo": np.ascontiguousarray(wo_pre).astype(NPDT),
            "masks": mask.astype(NPDT),
        })
    return in_maps


_NC = None


def _get_nc():
    global _NC
    if _NC is None:
        _NC = _build()
    return _NC


def run(hidden_states, Wq, Wk, Wv, Wo, trace=False, **trace_kwargs):
    nc = _get_nc()
    in_maps = _host_inputs(hidden_states, Wq, Wk, Wv, Wo)
    res = run_bass_kernel_spmd(nc, in_maps, core_ids=list(range(8)),
                               trace=trace, **trace_kwargs)
    out = np.zeros((B, L, D), dtype=np.float32)
    for core in range(8):
        out[core // 4] += res.results[core]["out"].astype(np.float32)
    return out, res


def kernel(hidden_states, Wq, Wk, Wv, Wo):
    out, _ = run(np.asarray(hidden_states, dtype=np.float32),
                 np.asarray(Wq, dtype=np.float32),
                 np.asarray(Wk, dtype=np.float32),
                 np.asarray(Wv, dtype=np.float32),
                 np.asarray(Wo, dtype=np.float32))
    return out


# revision 38
# speedup vs baseline: 1.0011x; 1.0011x over previous
"""Based linear-attention (parallel form) on 8 TRN2 NeuronCores.

Sharding: core c handles batch b = c // 4 and head-group g = c % 4
(3 of 12 heads).  Wq/Wk/Wv column-split by head, Wo row-split; each
core emits a partial [L, D] output and the host sums the 4 partials per
batch as part of unsharding.

v1 restructure vs v0 baseline (224us):
  - host pre-swizzles every weight into its SBUF layout so each input is
    ONE dma_start (issue cost ~0.6us each on the sync queue; v0 spent
    86us issuing 140 descriptors)
  - fused per-strip pipeline: V-proj group j -> attention strip j ->
    normalize -> output projection + store, all interleaved so PE never
    waits on a phase boundary and ACT/DVE copies spread across the run
  - ones row appended to qT/kT (K=17) so the PE emits s+1 directly;
    squares then split ACT (Square) / DVE (tensor_mul) per head to
    balance the two elementwise engines
  - off-diagonal tiles drop the "+1" fixup pass entirely: attn=(s+1)^2
    and the missing +1 is folded in as (o += cumsative v sums) via the
    normalize STT's per-partition scalar, and z += 128*(#full tiles)
  - z normalizer still via ones-lhsT matmuls (col-tiled across heads),
    1/z broadcast via K=1 matmul, normalize+drain fused in one DVE STT
"""

import sys

sys.path.insert(0, "/opt/trn_rl_repo")

from contextlib import ExitStack

import ml_dtypes
import numpy as np

import concourse.bass as bass
import concourse.tile as tile
from concourse import bacc, mybir
from concourse.bass_utils import run_bass_kernel_spmd

B, L, D = 2, 2048, 1536
H, FDIM, HD = 12, 16, 128
NH = 3          # heads per core
GQK = 96        # padded q/k rows (3 heads x 32; rows 32h..32h+15 = head,
                # row 32h+16 = ones so the K=17 matmul emits s+1)
DV = NH * HD    # 384 v/o columns per core
SW = 512        # l-strip width
P = 128
NK = D // P     # 12 contraction tiles
NM = L // P     # 16 m/l tiles
NJ = L // SW    # 4 l strips
NDC = D // SW   # 3 output column strips

DT = mybir.dt.bfloat16
NPDT = ml_dtypes.bfloat16
F32 = mybir.dt.float32
FP16 = mybir.dt.float16

_ADD = mybir.AluOpType.add
_MULT = mybir.AluOpType.mult
_SQUARE = mybir.ActivationFunctionType.Square
_COPY = mybir.ActivationFunctionType.Copy


def _build():
    nc = bacc.Bacc("TRN2", target_bir_lowering=False, debug=False, num_devices=8)

    hsT = nc.dram_tensor("hsT", [D, L], DT, kind="ExternalInput").ap()
    wq = nc.dram_tensor("wq", [P, NK * GQK], DT, kind="ExternalInput").ap()
    wk = nc.dram_tensor("wk", [P, NK * GQK], DT, kind="ExternalInput").ap()
    wv = nc.dram_tensor("wv", [P, NK * DV], DT, kind="ExternalInput").ap()
    wo = nc.dram_tensor("wo", [P, NH * D], DT, kind="ExternalInput").ap()
    masks = nc.dram_tensor("masks", [P, NJ * SW], DT, kind="ExternalInput").ap()
    out = nc.dram_tensor("out", [L, D], DT, kind="ExternalOutput").ap()

    with tile.TileContext(nc, trace_sim=False) as tc, ExitStack() as ctx:
        cpool = ctx.enter_context(tc.tile_pool(name="consts", bufs=1))
        wq_sb = cpool.tile([P, NK * GQK], DT, tag="wq")
        wk_sb = cpool.tile([P, NK * GQK], DT, tag="wk")
        wv_sb = cpool.tile([P, NK * DV], DT, tag="wv")
        wo_sb = cpool.tile([P, NH * D], DT, tag="wo")
        masks_sb = cpool.tile([P, NJ * SW], DT, tag="masks")
        ones_col = cpool.tile([P, 1], DT, tag="ones_col")
        ones_bc = cpool.tile([GQK, P], DT, tag="ones_bc")
        ones_rhs = cpool.tile([P, SW], DT, tag="ones_rhs")
        esel = cpool.tile([P, GQK], DT, tag="esel")
        vcum_sb = cpool.tile([P, NH], F32, tag="vcum")
        qT_sb = cpool.tile([GQK, L], DT, tag="qT")
        kT_sb = cpool.tile([GQK, L], DT, tag="kT")
        v_sb = cpool.tile([P, NM * DV], DT, tag="v")
        oT_sb = [cpool.tile([P, L], DT, tag=f"oT{h}", name=f"oT{h}") for h in range(NH)]
        zr_sb = cpool.tile([GQK, L], DT, tag="zr")
        zr32 = cpool.tile([GQK, SW], F32, tag="zr32")
        zsum_sb = cpool.tile([GQK, SW], F32, tag="zsum")
        bc_sb = [cpool.tile([P, SW], DT, tag=f"bc{h}", name=f"bc{h}") for h in range(NH)]
        hpool = ctx.enter_context(tc.tile_pool(name="hsT", bufs=NK))
        hs_t = [hpool.tile([P, L], DT, tag="hsT", name=f"hsT{k}") for k in range(NK)]

        # ---- input DMAs: one per tensor, hsT per k-strip for pipelining ----
        nc.sync.dma_start(wq_sb[:], wq[:])
        nc.sync.dma_start(hs_t[0][:], hsT[0:P, :])
        nc.sync.dma_start(wk_sb[:], wk[:])
        for k in range(1, 4):
            nc.sync.dma_start(hs_t[k][:], hsT[k * P : (k + 1) * P, :])
        nc.sync.dma_start(wv_sb[:], wv[:])
        for k in range(4, NK):
            nc.sync.dma_start(hs_t[k][:], hsT[k * P : (k + 1) * P, :])
        nc.sync.dma_start(masks_sb[:], masks[:])
        nc.sync.dma_start(wo_sb[:], wo[:])
        nc.gpsimd.memset(ones_col[:], 1.0)
        nc.gpsimd.memset(ones_bc[:], 1.0)
        nc.gpsimd.memset(vcum_sb[:], 0.0)
        nc.gpsimd.memset(ones_rhs[:], 1.0)
        # E-selector: E[0, 32h] = 1 -> an extra accumulating matmul
        # E.T @ ones sets qT/kT row 32h to 1.0 (the K=17 "s+1" trick)
        # without a slow single-partition memset after the drains
        nc.gpsimd.memset(esel[:], 0.0)
        for h in range(NH):
            nc.gpsimd.memset(esel[0:1, 32 * h : 32 * h + 1], 1.0)

        # ---- fused pipeline; QK proj shares the strip pools' psum so the
        #      attention matmuls aren't gated on a pool-boundary barrier ----
        with tc.tile_pool(name="ps_sq", bufs=NH, space="PSUM") as ps_sq, \
             tc.tile_pool(name="ps_acc", bufs=NH, space="PSUM") as ps_acc, \
             tc.tile_pool(name="ps_z", bufs=1, space="PSUM") as ps_z, \
             tc.tile_pool(name="ps_x", bufs=1, space="PSUM") as ps_x, \
             tc.tile_pool(name="attnT", bufs=15) as apool, \
             tc.tile_pool(name="obuf", bufs=3) as obuf:
            # -- Q/K projections, k-outer so matmuls chase the hsT DMAs --
            qps = [ps_sq.tile([GQK, SW], F32, tag="s", name=f"qp{j}") for j in range(NH)]
            qps.append(ps_z.tile([GQK, SW], F32, tag="z", name="qp3"))
            kps = [ps_acc.tile([GQK, SW], F32, tag="o", name=f"kp{j}") for j in range(NH)]
            kps.append(ps_x.tile([GQK, SW], F32, tag="x", name="kp3"))
            for k in range(NK):
                for j in range(NJ):
                    nc.tensor.matmul(
                        qps[j][:], wq_sb[:, k * GQK : (k + 1) * GQK],
                        hs_t[k][:, j * SW : (j + 1) * SW],
                        start=(k == 0), stop=False)
                for j in range(NJ):
                    nc.tensor.matmul(
                        kps[j][:], wk_sb[:, k * GQK : (k + 1) * GQK],
                        hs_t[k][:, j * SW : (j + 1) * SW],
                        start=(k == 0), stop=False)
            for j in range(NJ):
                nc.tensor.matmul(qps[j][:], esel[:], ones_rhs[:], start=False, stop=True)
                nc.tensor.matmul(kps[j][:], esel[:], ones_rhs[:], start=False, stop=True)
            # drain order frees the x- and z-banks first (they double-buffer
            # V-proj), then s (sT tiles); alternate ACT/DVE to halve latency
            drains = [(kps[3], kT_sb, 3), (qps[3], qT_sb, 3), (qps[0], qT_sb, 0),
                      (qps[1], qT_sb, 1), (qps[2], qT_sb, 2), (kps[0], kT_sb, 0),
                      (kps[1], kT_sb, 1), (kps[2], kT_sb, 2)]
            for idx, (src, dst, jj) in enumerate(drains):
                if idx % 2 == 0:
                    nc.scalar.copy(dst[:, jj * SW : (jj + 1) * SW], src[:])
                else:
                    nc.vector.tensor_copy(dst[:, jj * SW : (jj + 1) * SW], src[:])

            def emit_vproj(g):
                # double-buffer across the x- and z-rings (z is idle during
                # V-proj) so vp never waits on its own drain
                for n, mt in enumerate(range(4 * g, 4 * g + 4)):
                    pool, tg = (ps_x, "x") if n % 2 == 0 else (ps_z, "z")
                    vp = pool.tile([P, SW], F32, tag=tg, name=f"vp{mt}")
                    for k in range(NK):
                        nc.tensor.matmul(
                            vp[:, :DV], hs_t[k][:, mt * P : (mt + 1) * P],
                            wv_sb[:, k * DV : (k + 1) * DV],
                            start=(k == 0), stop=(k == NK - 1))
                    nc.vector.tensor_copy(v_sb[:, mt * DV : (mt + 1) * DV], vp[:, :DV])

            # oproj work from strip j is deferred and interleaved into strip
            # j+1's attention stream as PE ballast: it has no elementwise
            # dependencies, so it fills the PE's dependency micro-gaps and
            # keeps the HAM clock-gate at full rate (idle windows re-throttle
            # the PE to 1.2 GHz for ~7-14us at a time, measured)
            obtiles = {}

            def make_oproj_group(lt, dc, flip):
                def emit():
                    ops = ps_x.tile([P, SW], F32, tag="x", name=f"op{lt}_{dc}")
                    for h in range(NH):
                        nc.tensor.matmul(
                            ops[:], oT_sb[h][:, lt * P : (lt + 1) * P],
                            wo_sb[:, h * D + dc * SW : h * D + (dc + 1) * SW],
                            start=(h == 0), stop=(h == NH - 1))
                    if dc == 0:
                        obtiles[lt] = obuf.tile([P, D], DT, tag="ob", name=f"ob{lt}")
                    ob = obtiles[lt]
                    if flip:
                        nc.scalar.copy(ob[:, dc * SW : (dc + 1) * SW], ops[:])
                    else:
                        nc.vector.tensor_copy(ob[:, dc * SW : (dc + 1) * SW], ops[:])
                    if dc == NDC - 1:
                        nc.sync.dma_start(out[lt * P : (lt + 1) * P, :], ob[:])
                return emit

            ballast = []

            for j in range(NJ):
                if j == 0:
                    emit_vproj(0)

                # -- (b) attention strip j --
                nim = 4 * (j + 1)
                otp = [ps_acc.tile([P, SW], F32, tag="o", name=f"otp{j}_{h}")
                       for h in range(NH)]
                zp = ps_z.tile([GQK, SW], F32, tag="z", name=f"zp{j}")

                def emit_av(im, atts, j=j, nim=nim, otp=otp, zp=zp):
                    c = im - 4 * j
                    f0 = max(c, 0) * P
                    for h in range(NH):
                        nc.tensor.matmul(
                            otp[h][:, f0:SW],
                            v_sb[:, im * DV + h * HD : im * DV + (h + 1) * HD],
                            atts[h][:, f0:SW], start=(im == 0), stop=(im == nim - 1))
                    for h in range(NH):
                        nc.tensor.matmul(
                            zp[32 * h : 32 * h + 1, f0:SW], ones_col[:], atts[h][:, f0:SW],
                            start=(im == 0), stop=(im == nim - 1))

                pending = []   # (im, atts) awaiting AV/z; depth-2 so the PE
                               # never dispatch-stalls on a square in flight
                nsq = 0
                for im in range(nim):
                    c = im - 4 * j
                    f0 = max(c, 0) * P
                    cur = []
                    for h in range(NH):
                        r0 = 32 * h
                        stp = ps_sq.tile([P, SW], F32, tag="s",
                                        name=f"stp{j}_{im}_{h}")
                        nsq += 1
                        nc.tensor.matmul(
                            stp[:, f0:SW], kT_sb[r0 : r0 + FDIM + 1, im * P : (im + 1) * P],
                            qT_sb[r0 : r0 + FDIM + 1, j * SW + f0 : (j + 1) * SW],
                            start=True, stop=True)
                        att = apool.tile([P, SW], DT, tag="a")
                        if c >= 0:
                            # diag: ((s+1)^2 + 1) * mask   (ACT square, DVE fixup)
                            nc.scalar.activation(att[:, f0:SW], stp[:, f0:SW], _SQUARE)
                            nc.vector.scalar_tensor_tensor(
                                att[:, f0:SW], att[:, f0:SW], 1.0,
                                masks_sb[:, c * SW + f0 : (c + 1) * SW],
                                op0=_ADD, op1=_MULT)
                        elif h == 1:
                            # off-diag on DVE (drain cast + in-place square)
                            # to relieve ACT, the attention-loop bottleneck
                            nc.vector.tensor_copy(att[:], stp[:])
                            nc.vector.tensor_mul(att[:], att[:], att[:])
                        else:
                            # off-diag: (s+1)^2, +1 folded via vcum/z-const
                            nc.scalar.activation(att[:], stp[:], _SQUARE)
                        cur.append(att)
                    pending.append((im, cur))
                    if len(pending) > 2:
                        pim, patts = pending.pop(0)
                        emit_av(pim, patts)
                    # PE ballast: prior strip's oproj groups, ONE per slot so
                    # the group's psum->sbuf copy lands a full slot before the
                    # next group's matmul reuses the bank (no HOL stall);
                    # start at slot 2 so the normalize STTs have completed
                    if im >= 2 and ballast:
                        ballast.pop(0)()
                for pim, patts in pending:
                    emit_av(pim, patts)
                while ballast:
                    ballast.pop(0)()

                # -- (c) normalizer + per-strip normalize --
                if j > 0:
                    # off-diag tiles' missing "+1": z gains 128 per full tile
                    nc.vector.tensor_scalar_add(zsum_sb[:], zp[:], float(P * 4 * j))
                    nc.vector.reciprocal_approx_fast(zr32[:], zsum_sb[:])
                else:
                    nc.vector.reciprocal_approx_fast(zr32[:], zp[:])
                with nc.allow_low_precision(reason="1/z in fp16 is plenty for 2e-2 gate"):
                    nc.vector.tensor_copy(zr_sb[:, j * SW : (j + 1) * SW], zr32[:])
                # next strip's V-proj keeps PE busy under the recip chain
                if j < NJ - 1:
                    emit_vproj(j + 1)
                for h in range(NH):
                    r0 = 32 * h
                    bc = ps_x.tile([P, SW], F32, tag="x", name=f"bc{j}_{h}")
                    nc.tensor.matmul(
                        bc[:], ones_bc[r0 : r0 + 1, :],
                        zr_sb[r0 : r0 + 1, j * SW : (j + 1) * SW],
                        start=True, stop=True)
                    bcs = bc_sb[h]
                    nc.vector.tensor_copy(bcs[:], bc[:])
                    # oT = (otp + vcum) * bc  : drain+(+1-fold)+normalize in one op
                    nc.vector.scalar_tensor_tensor(
                        oT_sb[h][:, j * SW : (j + 1) * SW], otp[h][:],
                        vcum_sb[:, h : h + 1], bcs[:], op0=_ADD, op1=_MULT)
                # vcum += column sums of this strip's v tiles (for NEXT strips)
                if j < NJ - 1:
                    vps = ps_x.tile([P, SW], F32, tag="x", name=f"vps{j}")
                    # accumulation groups must be sequential per PSUM bank:
                    # interleaving start/stop groups at different free offsets
                    # corrupts the accumulation (HW-verified)
                    for h in range(NH):
                        for mt in range(4 * j, 4 * j + 4):
                            nc.tensor.matmul(
                                vps[:, h : h + 1],
                                v_sb[:, mt * DV + h * HD : mt * DV + (h + 1) * HD],
                                ones_col[:], start=(mt == 4 * j), stop=(mt == 4 * j + 3))
                    nc.vector.tensor_add(vcum_sb[:], vcum_sb[:], vps[:, :NH])

                # -- (d) output projection: deferred into strip j+1's stream --
                flip = True
                for lt in range(4 * j, 4 * j + 4):
                    for dc in range(NDC):
                        ballast.append(make_oproj_group(lt, dc, flip))
                        flip = not flip
                if j == NJ - 1:
                    while ballast:
                        ballast.pop(0)()

    nc.compile()
    return nc


def _host_inputs(hidden_states, Wq, Wk, Wv, Wo):
    """Shard + lay out the full inputs into 8 per-core in_maps.

    All weights are pre-swizzled to their SBUF layouts so each is one DMA.
    """
    scale = FDIM ** -0.5
    mask = np.zeros((P, NJ * SW), dtype=np.float32)
    p = np.arange(P)[:, None]
    f = np.arange(SW)[None, :]
    for c in range(NJ):
        mask[:, c * SW : (c + 1) * SW] = (p + P * c <= f).astype(np.float32)

    in_maps = []
    for core in range(8):
        b, g = divmod(core, 4)
        heads = range(NH * g, NH * (g + 1))
        # [P, NK*GQK]: chunk k holds Wq[128k:128k+128, :] with head h at
        # columns 32h+1..32h+16 (rest zero; row 32h seeded to 1 on device
        # for the K=17 "s+1" trick — 1-partition ops need 32-aligned base)
        wq_pre = np.zeros((P, NK * GQK), dtype=np.float32)
        wk_pre = np.zeros((P, NK * GQK), dtype=np.float32)
        for k in range(NK):
            for i, h in enumerate(heads):
                wq_pre[:, k * GQK + 32 * i + 1 : k * GQK + 32 * i + 1 + FDIM] = \
                    Wq[k * P : (k + 1) * P, FDIM * h : FDIM * (h + 1)] * scale
                wk_pre[:, k * GQK + 32 * i + 1 : k * GQK + 32 * i + 1 + FDIM] = \
                    Wk[k * P : (k + 1) * P, FDIM * h : FDIM * (h + 1)]
        # [P, NK*DV]: chunk k = Wv[128k:128k+128, group cols]
        wv_pre = Wv[:, DV * g : DV * (g + 1)].reshape(NK, P, DV)
        wv_pre = wv_pre.transpose(1, 0, 2).reshape(P, NK * DV)
        # [P, NH*D]: chunk h = Wo[384g+128h : +128, :]
        wo_pre = Wo[DV * g : DV * (g + 1), :].reshape(NH, P, D)
        wo_pre = wo_pre.transpose(1, 0, 2).reshape(P, NH * D)
        in_maps.append({
            "hsT": np.ascontiguousarray(hidden_states[b].T).astype(NPDT),
            "wq": wq_pre.astype(NPDT),
            "wk": wk_pre.astype(NPDT),
            "wv": np.ascontiguousarray(wv_pre).astype(NPDT),
            "wo": np.ascontiguousarray(wo_pre).astype(NPDT),
            "masks": mask.astype(NPDT),
        })
    return in_maps


_NC = None


def _get_nc():
    global _NC
    if _NC is None:
        _NC = _build()
    return _NC


def run(hidden_states, Wq, Wk, Wv, Wo, trace=False, **trace_kwargs):
    nc = _get_nc()
    in_maps = _host_inputs(hidden_states, Wq, Wk, Wv, Wo)
    res = run_bass_kernel_spmd(nc, in_maps, core_ids=list(range(8)),
                               trace=trace, **trace_kwargs)
    out = np.zeros((B, L, D), dtype=np.float32)
    for core in range(8):
        out[core // 4] += res.results[core]["out"].astype(np.float32)
    return out, res


def kernel(hidden_states, Wq, Wk, Wv, Wo):
    out, _ = run(np.asarray(hidden_states, dtype=np.float32),
                 np.asarray(Wq, dtype=np.float32),
                 np.asarray(Wk, dtype=np.float32),
                 np.asarray(Wv, dtype=np.float32),
                 np.asarray(Wo, dtype=np.float32))
    return out


# revision 40
# speedup vs baseline: 1.3210x; 1.3196x over previous
"""Based linear-attention (parallel form) on 8 TRN2 NeuronCores.

Sharding: core c handles batch b = c // 4 and head-group g = c % 4
(3 of 12 heads).  Wq/Wk/Wv column-split by head, Wo row-split; each
core emits a partial [L, D] output and the host sums the 4 partials per
batch as part of unsharding.

v1 restructure vs v0 baseline (224us):
  - host pre-swizzles every weight into its SBUF layout so each input is
    ONE dma_start (issue cost ~0.6us each on the sync queue; v0 spent
    86us issuing 140 descriptors)
  - fused per-strip pipeline: V-proj group j -> attention strip j ->
    normalize -> output projection + store, all interleaved so PE never
    waits on a phase boundary and ACT/DVE copies spread across the run
  - ones row appended to qT/kT (K=17) so the PE emits s+1 directly;
    squares then split ACT (Square) / DVE (tensor_mul) per head to
    balance the two elementwise engines
  - off-diagonal tiles drop the "+1" fixup pass entirely: attn=(s+1)^2
    and the missing +1 is folded in as (o += cumsative v sums) via the
    normalize STT's per-partition scalar, and z += 128*(#full tiles)
  - z normalizer still via ones-lhsT matmuls (col-tiled across heads),
    1/z broadcast via K=1 matmul, normalize+drain fused in one DVE STT
"""

import sys

sys.path.insert(0, "/opt/trn_rl_repo")

from contextlib import ExitStack

import ml_dtypes
import numpy as np

import concourse.bass as bass
import concourse.tile as tile
from concourse import bacc, mybir
from concourse.bass_utils import run_bass_kernel_spmd

B, L, D = 2, 2048, 1536
H, FDIM, HD = 12, 16, 128
NH = 3          # heads per core
GQK = 96        # padded q/k rows (3 heads x 32; rows 32h..32h+15 = head,
                # row 32h+16 = ones so the K=17 matmul emits s+1)
DV = NH * HD    # 384 v/o columns per core
SW = 512        # l-strip width
P = 128
NK = D // P     # 12 contraction tiles
NM = L // P     # 16 m/l tiles
NJ = L // SW    # 4 l strips
NDC = D // SW   # 3 output column strips

DT = mybir.dt.bfloat16
NPDT = ml_dtypes.bfloat16
F32 = mybir.dt.float32
FP16 = mybir.dt.float16

_ADD = mybir.AluOpType.add
_MULT = mybir.AluOpType.mult
_SQUARE = mybir.ActivationFunctionType.Square
_COPY = mybir.ActivationFunctionType.Copy


def _build():
    nc = bacc.Bacc("TRN2", target_bir_lowering=False, debug=False, num_devices=8)

    hsT = nc.dram_tensor("hsT", [D, L], DT, kind="ExternalInput").ap()
    wq = nc.dram_tensor("wq", [P, NK * GQK], DT, kind="ExternalInput").ap()
    wk = nc.dram_tensor("wk", [P, NK * GQK], DT, kind="ExternalInput").ap()
    wv = nc.dram_tensor("wv", [P, NK * DV], DT, kind="ExternalInput").ap()
    wo = nc.dram_tensor("wo", [P, NH * D], DT, kind="ExternalInput").ap()
    masks = nc.dram_tensor("masks", [P, NJ * SW], DT, kind="ExternalInput").ap()
    out = nc.dram_tensor("out", [L, D], DT, kind="ExternalOutput").ap()

    with tile.TileContext(nc, trace_sim=False) as tc, ExitStack() as ctx:
        cpool = ctx.enter_context(tc.tile_pool(name="consts", bufs=1))
        wq_sb = cpool.tile([P, NK * GQK], DT, tag="wq")
        wk_sb = cpool.tile([P, NK * GQK], DT, tag="wk")
        wv_sb = cpool.tile([P, NK * DV], DT, tag="wv")
        wo_sb = cpool.tile([P, NH * D], DT, tag="wo")
        masks_sb = cpool.tile([P, NJ * SW], DT, tag="masks")
        ones_col = cpool.tile([P, 1], DT, tag="ones_col")
        ones_bc = cpool.tile([GQK, P], DT, tag="ones_bc")
        ones_rhs = cpool.tile([P, SW], DT, tag="ones_rhs")
        esel = cpool.tile([P, GQK], DT, tag="esel")
        vcum_sb = cpool.tile([P, NH], F32, tag="vcum")
        qT_sb = cpool.tile([GQK, L], DT, tag="qT")
        kT_sb = cpool.tile([GQK, L], DT, tag="kT")
        v_sb = cpool.tile([P, NM * DV], DT, tag="v")
        oT_sb = [cpool.tile([P, L], DT, tag=f"oT{h}", name=f"oT{h}") for h in range(NH)]
        zr_sb = cpool.tile([GQK, L], DT, tag="zr")
        zr32 = cpool.tile([GQK, SW], F32, tag="zr32")
        zsum_sb = cpool.tile([GQK, SW], F32, tag="zsum")
        bc_sb = [cpool.tile([P, SW], DT, tag=f"bc{h}", name=f"bc{h}") for h in range(NH)]
        hpool = ctx.enter_context(tc.tile_pool(name="hsT", bufs=NK))
        hs_t = [hpool.tile([P, L], DT, tag="hsT", name=f"hsT{k}") for k in range(NK)]

        # ---- input DMAs: one per tensor, hsT per k-strip for pipelining ----
        nc.sync.dma_start(wq_sb[:], wq[:])
        nc.sync.dma_start(hs_t[0][:], hsT[0:P, :])
        nc.sync.dma_start(wk_sb[:], wk[:])
        for k in range(1, 4):
            nc.sync.dma_start(hs_t[k][:], hsT[k * P : (k + 1) * P, :])
        nc.sync.dma_start(wv_sb[:], wv[:])
        for k in range(4, NK):
            nc.sync.dma_start(hs_t[k][:], hsT[k * P : (k + 1) * P, :])
        nc.sync.dma_start(masks_sb[:], masks[:])
        nc.sync.dma_start(wo_sb[:], wo[:])
        nc.gpsimd.memset(ones_col[:], 1.0)
        nc.gpsimd.memset(ones_bc[:], 1.0)
        nc.gpsimd.memset(vcum_sb[:], 0.0)
        nc.gpsimd.memset(ones_rhs[:], 1.0)
        # E-selector: E[0, 32h] = 1 -> an extra accumulating matmul
        # E.T @ ones sets qT/kT row 32h to 1.0 (the K=17 "s+1" trick)
        # without a slow single-partition memset after the drains
        nc.gpsimd.memset(esel[:], 0.0)
        for h in range(NH):
            nc.gpsimd.memset(esel[0:1, 32 * h : 32 * h + 1], 1.0)

        # ---- fused pipeline; QK proj shares the strip pools' psum so the
        #      attention matmuls aren't gated on a pool-boundary barrier ----
        with tc.tile_pool(name="ps_sq", bufs=NH, space="PSUM") as ps_sq, \
             tc.tile_pool(name="ps_acc", bufs=NH, space="PSUM") as ps_acc, \
             tc.tile_pool(name="ps_z", bufs=1, space="PSUM") as ps_z, \
             tc.tile_pool(name="ps_x", bufs=1, space="PSUM") as ps_x, \
             tc.tile_pool(name="attnT", bufs=15) as apool, \
             tc.tile_pool(name="obuf", bufs=3) as obuf:
            # -- Q/K projections, k-outer so matmuls chase the hsT DMAs --
            qps = [ps_sq.tile([GQK, SW], F32, tag="s", name=f"qp{j}") for j in range(NH)]
            qps.append(ps_z.tile([GQK, SW], F32, tag="z", name="qp3"))
            kps = [ps_acc.tile([GQK, SW], F32, tag="o", name=f"kp{j}") for j in range(NH)]
            kps.append(ps_x.tile([GQK, SW], F32, tag="x", name="kp3"))
            for k in range(NK):
                for j in range(NJ):
                    nc.tensor.matmul(
                        qps[j][:], wq_sb[:, k * GQK : (k + 1) * GQK],
                        hs_t[k][:, j * SW : (j + 1) * SW],
                        start=(k == 0), stop=False)
                for j in range(NJ):
                    nc.tensor.matmul(
                        kps[j][:], wk_sb[:, k * GQK : (k + 1) * GQK],
                        hs_t[k][:, j * SW : (j + 1) * SW],
                        start=(k == 0), stop=False)
            for j in range(NJ):
                nc.tensor.matmul(qps[j][:], esel[:], ones_rhs[:], start=False, stop=True)
                nc.tensor.matmul(kps[j][:], esel[:], ones_rhs[:], start=False, stop=True)
            # drain order frees the x- and z-banks first (they double-buffer
            # V-proj), then s (sT tiles); alternate ACT/DVE to halve latency
            drains = [(kps[3], kT_sb, 3), (qps[3], qT_sb, 3), (qps[0], qT_sb, 0),
                      (qps[1], qT_sb, 1), (qps[2], qT_sb, 2), (kps[0], kT_sb, 0),
                      (kps[1], kT_sb, 1), (kps[2], kT_sb, 2)]
            for idx, (src, dst, jj) in enumerate(drains):
                if idx % 2 == 0:
                    nc.scalar.copy(dst[:, jj * SW : (jj + 1) * SW], src[:])
                else:
                    nc.vector.tensor_copy(dst[:, jj * SW : (jj + 1) * SW], src[:])

            def emit_vproj(g):
                # double-buffer across the x- and z-rings (z is idle during
                # V-proj) so vp never waits on its own drain
                for n, mt in enumerate(range(4 * g, 4 * g + 4)):
                    pool, tg = (ps_x, "x") if n % 2 == 0 else (ps_z, "z")
                    vp = pool.tile([P, SW], F32, tag=tg, name=f"vp{mt}")
                    for k in range(NK):
                        nc.tensor.matmul(
                            vp[:, :DV], hs_t[k][:, mt * P : (mt + 1) * P],
                            wv_sb[:, k * DV : (k + 1) * DV],
                            start=(k == 0), stop=(k == NK - 1))
                    nc.vector.tensor_copy(v_sb[:, mt * DV : (mt + 1) * DV], vp[:, :DV])

            # oproj work from strip j is deferred and interleaved into strip
            # j+1's attention stream as PE ballast: it has no elementwise
            # dependencies, so it fills the PE's dependency micro-gaps and
            # keeps the HAM clock-gate at full rate (idle windows re-throttle
            # the PE to 1.2 GHz for ~7-14us at a time, measured)
            obtiles = {}

            def make_oproj_group(lt, dc, flip, pool=None, tg="x"):
                def emit():
                    ops = (pool or ps_x).tile([P, SW], F32, tag=tg, name=f"op{lt}_{dc}")
                    for h in range(NH):
                        nc.tensor.matmul(
                            ops[:], oT_sb[h][:, lt * P : (lt + 1) * P],
                            wo_sb[:, h * D + dc * SW : h * D + (dc + 1) * SW],
                            start=(h == 0), stop=(h == NH - 1))
                    if dc == 0:
                        obtiles[lt] = obuf.tile([P, D], DT, tag="ob", name=f"ob{lt}")
                    ob = obtiles[lt]
                    if flip:
                        nc.scalar.copy(ob[:, dc * SW : (dc + 1) * SW], ops[:])
                    else:
                        nc.vector.tensor_copy(ob[:, dc * SW : (dc + 1) * SW], ops[:])
                    if dc == NDC - 1:
                        nc.sync.dma_start(out[lt * P : (lt + 1) * P, :], ob[:])
                return emit

            ballast = []

            for j in range(NJ):
                if j == 0:
                    emit_vproj(0)

                # -- (b) attention strip j --
                nim = 4 * (j + 1)
                otp = [ps_acc.tile([P, SW], F32, tag="o", name=f"otp{j}_{h}")
                       for h in range(NH)]
                zp = ps_z.tile([GQK, SW], F32, tag="z", name=f"zp{j}")

                def emit_av(im, atts, j=j, nim=nim, otp=otp, zp=zp):
                    c = im - 4 * j
                    f0 = max(c, 0) * P
                    for h in range(NH):
                        nc.tensor.matmul(
                            otp[h][:, f0:SW],
                            v_sb[:, im * DV + h * HD : im * DV + (h + 1) * HD],
                            atts[h][:, f0:SW], start=(im == 0), stop=(im == nim - 1))
                    for h in range(NH):
                        nc.tensor.matmul(
                            zp[32 * h : 32 * h + 1, f0:SW], ones_col[:], atts[h][:, f0:SW],
                            start=(im == 0), stop=(im == nim - 1))

                pending = []   # (im, atts) awaiting AV/z; depth-2 so the PE
                               # never dispatch-stalls on a square in flight
                nsq = 0
                for im in range(nim):
                    c = im - 4 * j
                    f0 = max(c, 0) * P
                    cur = []
                    for h in range(NH):
                        r0 = 32 * h
                        stp = ps_sq.tile([P, SW], F32, tag="s",
                                        name=f"stp{j}_{im}_{h}")
                        nsq += 1
                        nc.tensor.matmul(
                            stp[:, f0:SW], kT_sb[r0 : r0 + FDIM + 1, im * P : (im + 1) * P],
                            qT_sb[r0 : r0 + FDIM + 1, j * SW + f0 : (j + 1) * SW],
                            start=True, stop=True)
                        att = apool.tile([P, SW], DT, tag="a")
                        if c >= 0:
                            # diag: ((s+1)^2 + 1) * mask   (ACT square, DVE fixup)
                            nc.scalar.activation(att[:, f0:SW], stp[:, f0:SW], _SQUARE)
                            nc.vector.scalar_tensor_tensor(
                                att[:, f0:SW], att[:, f0:SW], 1.0,
                                masks_sb[:, c * SW + f0 : (c + 1) * SW],
                                op0=_ADD, op1=_MULT)
                        elif h == 1:
                            # off-diag on DVE (drain cast + in-place square)
                            # to relieve ACT, the attention-loop bottleneck
                            nc.vector.tensor_copy(att[:], stp[:])
                            nc.vector.tensor_mul(att[:], att[:], att[:])
                        else:
                            # off-diag: (s+1)^2, +1 folded via vcum/z-const
                            nc.scalar.activation(att[:], stp[:], _SQUARE)
                        cur.append(att)
                    pending.append((im, cur))
                    if len(pending) > 2:
                        pim, patts = pending.pop(0)
                        emit_av(pim, patts)
                    # PE ballast: prior strip's oproj groups, ONE per slot so
                    # the group's psum->sbuf copy lands a full slot before the
                    # next group's matmul reuses the bank (no HOL stall);
                    # start at slot 2 so the normalize STTs have completed
                    if im >= 2 and ballast:
                        ballast.pop(0)()
                for pim, patts in pending:
                    emit_av(pim, patts)
                while ballast:
                    ballast.pop(0)()

                # -- (c) normalizer + per-strip normalize --
                if j > 0:
                    # off-diag tiles' missing "+1": z gains 128 per full tile
                    nc.vector.tensor_scalar_add(zsum_sb[:], zp[:], float(P * 4 * j))
                    nc.vector.reciprocal_approx_fast(zr32[:], zsum_sb[:])
                else:
                    nc.vector.reciprocal_approx_fast(zr32[:], zp[:])
                with nc.allow_low_precision(reason="1/z in fp16 is plenty for 2e-2 gate"):
                    nc.vector.tensor_copy(zr_sb[:, j * SW : (j + 1) * SW], zr32[:])
                # next strip's V-proj keeps PE busy under the recip chain
                if j < NJ - 1:
                    emit_vproj(j + 1)
                for h in range(NH):
                    r0 = 32 * h
                    bc = ps_x.tile([P, SW], F32, tag="x", name=f"bc{j}_{h}")
                    nc.tensor.matmul(
                        bc[:], ones_bc[r0 : r0 + 1, :],
                        zr_sb[r0 : r0 + 1, j * SW : (j + 1) * SW],
                        start=True, stop=True)
                    bcs = bc_sb[h]
                    nc.vector.tensor_copy(bcs[:], bc[:])
                    # oT = (otp + vcum) * bc  : drain+(+1-fold)+normalize in one op
                    nc.vector.scalar_tensor_tensor(
                        oT_sb[h][:, j * SW : (j + 1) * SW], otp[h][:],
                        vcum_sb[:, h : h + 1], bcs[:], op0=_ADD, op1=_MULT)
                # vcum += column sums of this strip's v tiles (for NEXT strips)
                if j < NJ - 1:
                    vps = ps_x.tile([P, SW], F32, tag="x", name=f"vps{j}")
                    # accumulation groups must be sequential per PSUM bank:
                    # interleaving start/stop groups at different free offsets
                    # corrupts the accumulation (HW-verified)
                    for h in range(NH):
                        for mt in range(4 * j, 4 * j + 4):
                            nc.tensor.matmul(
                                vps[:, h : h + 1],
                                v_sb[:, mt * DV + h * HD : mt * DV + (h + 1) * HD],
                                ones_col[:], start=(mt == 4 * j), stop=(mt == 4 * j + 3))
                    nc.vector.tensor_add(vcum_sb[:], vcum_sb[:], vps[:, :NH])

                # -- (d) output projection: deferred into strip j+1's stream;
                #       the last strip's groups run as a tail block on the
                #       freed 3-deep s-ring (the single x-bank would idle PE) --
                flip = True
                for lt in range(4 * j, 4 * j + 4):
                    for dc in range(NDC):
                        if j == NJ - 1:
                            make_oproj_group(lt, dc, flip, pool=ps_sq, tg="s")()
                        else:
                            ballast.append(make_oproj_group(lt, dc, flip))
                        flip = not flip

    nc.compile()
    return nc


def _host_inputs(hidden_states, Wq, Wk, Wv, Wo):
    """Shard + lay out the full inputs into 8 per-core in_maps.

    All weights are pre-swizzled to their SBUF layouts so each is one DMA.
    """
    scale = FDIM ** -0.5
    mask = np.zeros((P, NJ * SW), dtype=np.float32)
    p = np.arange(P)[:, None]
    f = np.arange(SW)[None, :]
    for c in range(NJ):
        mask[:, c * SW : (c + 1) * SW] = (p + P * c <= f).astype(np.float32)

    in_maps = []
    for core in range(8):
        b, g = divmod(core, 4)
        heads = range(NH * g, NH * (g + 1))
        # [P, NK*GQK]: chunk k holds Wq[128k:128k+128, :] with head h at
        # columns 32h+1..32h+16 (rest zero; row 32h seeded to 1 on device
        # for the K=17 "s+1" trick — 1-partition ops need 32-aligned base)
        wq_pre = np.zeros((P, NK * GQK), dtype=np.float32)
        wk_pre = np.zeros((P, NK * GQK), dtype=np.float32)
        for k in range(NK):
            for i, h in enumerate(heads):
                wq_pre[:, k * GQK + 32 * i + 1 : k * GQK + 32 * i + 1 + FDIM] = \
                    Wq[k * P : (k + 1) * P, FDIM * h : FDIM * (h + 1)] * scale
                wk_pre[:, k * GQK + 32 * i + 1 : k * GQK + 32 * i + 1 + FDIM] = \
                    Wk[k * P : (k + 1) * P, FDIM * h : FDIM * (h + 1)]
        # [P, NK*DV]: chunk k = Wv[128k:128k+128, group cols]
        wv_pre = Wv[:, DV * g : DV * (g + 1)].reshape(NK, P, DV)
        wv_pre = wv_pre.transpose(1, 0, 2).reshape(P, NK * DV)
        # [P, NH*D]: chunk h = Wo[384g+128h : +128, :]
        wo_pre = Wo[DV * g : DV * (g + 1), :].reshape(NH, P, D)
        wo_pre = wo_pre.transpose(1, 0, 2).reshape(P, NH * D)
        in_maps.append({
            "hsT": np.ascontiguousarray(hidden_states[b].T).astype(NPDT),
            "wq": wq_pre.astype(NPDT),
            "wk": wk_pre.astype(NPDT),
            "wv": np.ascontiguousarray(wv_pre).astype(NPDT),
            "wo": np.ascontiguousarray(wo_pre).astype(NPDT),
            "masks": mask.astype(NPDT),
        })
    return in_maps


_NC = None


def _get_nc():
    global _NC
    if _NC is None:
        _NC = _build()
    return _NC


def run(hidden_states, Wq, Wk, Wv, Wo, trace=False, **trace_kwargs):
    nc = _get_nc()
    in_maps = _host_inputs(hidden_states, Wq, Wk, Wv, Wo)
    res = run_bass_kernel_spmd(nc, in_maps, core_ids=list(range(8)),
                               trace=trace, **trace_kwargs)
    out = np.zeros((B, L, D), dtype=np.float32)
    for core in range(8):
        out[core // 4] += res.results[core]["out"].astype(np.float32)
    return out, res


def kernel(hidden_states, Wq, Wk, Wv, Wo):
    out, _ = run(np.asarray(hidden_states, dtype=np.float32),
                 np.asarray(Wq, dtype=np.float32),
                 np.asarray(Wk, dtype=np.float32),
                 np.asarray(Wv, dtype=np.float32),
                 np.asarray(Wo, dtype=np.float32))
    return out
